# revision 7
# baseline (speedup 1.0000x reference)
"""Cross-attention Trainium2 kernel (8 NeuronCores, SPMD).

Sharding: core c handles batch c//2 and head-group c%2 (8 of 16 heads).
Each core computes its head-group's partial output projection; the host
sums the two partials per batch (bias is folded into head-group 0).

Shapes (hardcoded): B=4, N=2048 (queries), M=1024 (context), K=1024
(query/context dim), H=16 heads, DH=64, head-group width DHG=512, E=1024.

All operands are fp16 on-chip (PSUM accumulation stays fp32); numerics
validated at rel err ~5e-4 vs the fp32 reference (gate is 2e-2).

Per-core dataflow:
  x/ctx are DMA-transpose-loaded (XBAR) straight into k-major layout, so
  the PE does no transposes.  K.T = Wk.T @ ctxT, V = ctxT.T @ Wv (+ones
  col), Q.T = Wq.T @ xT.  Per (head, 512-query chunk): S.T = K.T_h.T @
  Q.T_h (m on partitions), P.T = exp(S.T * scale) via ACT -> fp16, then
  PV in the n-on-partitions orientation: O[n,dh] += P-chunk.T @ [V_h|1],
  which uses the full 128-partition output (half the PE rows of the
  dh-on-partitions orientation) and yields softmax row-sums in column 64.
  DVE normalizes O by the reciprocal row-sums while copying PSUM->SBUF.
  O is stored per head to a DRAM scratch and XBAR-transposed back to
  dhg-major for the output projection (the XBAR requires a DRAM source);
  the bias is added by DVE during the final PSUM->SBUF copy.  Output
  projection and next-chunk Q projection are emitted interleaved into
  the following chunk's attention so the PE counter the ACT engine
  waits on is not held back by them.
"""
import sys

if "/opt/trn_rl_repo" not in sys.path:
    sys.path.insert(0, "/opt/trn_rl_repo")

import numpy as np

import concourse.bass as bass  # noqa: F401
import concourse.tile as tile
from concourse import bacc, mybir
from concourse.bass_utils import run_bass_kernel_spmd

P = 128
N = 2048          # queries per batch
M = 1024          # context rows
K = 1024          # query_dim == context_dim
DHG = 512         # d_attn per head group (8 heads x 64)
DH = 64           # dim per head
HL = 8            # heads per core
E = 1024          # output dim
SCALE = DH ** -0.5
F32 = mybir.dt.float32
F16 = mybir.dt.float16

KO = K // P       # 8 contraction chunks
MT = M // P       # 8 context tiles
DO = DHG // P     # 4 head-dim chunks
QC = N // 512     # 4 query chunks of 512
NC = 512 // P     # 4 query sub-tiles per chunk
EC = E // 512     # 2 output chunks of 512

_CACHE = {}


def _build():
    nc = bacc.Bacc("TRN2", target_bir_lowering=False, debug=False, num_devices=8)
    x_d = nc.dram_tensor("x", [N, K], F16, kind="ExternalInput")
    ctx_d = nc.dram_tensor("ctx", [M, K], F16, kind="ExternalInput")
    wq_d = nc.dram_tensor("wq", [K, DHG], F16, kind="ExternalInput")
    wk_d = nc.dram_tensor("wk", [K, DHG], F16, kind="ExternalInput")
    wv_d = nc.dram_tensor("wv", [K, DHG], F16, kind="ExternalInput")
    wo_d = nc.dram_tensor("wo", [DHG, E], F16, kind="ExternalInput")
    bo_d = nc.dram_tensor("bo", [1, E], F32, kind="ExternalInput")
    out_d = nc.dram_tensor("out", [N, E], F16, kind="ExternalOutput")
    # DRAM scratch for the O round-trip: the XBAR transpose only works with
    # a DRAM source (SBUF->SBUF DMA transpose returns garbage on HW).
    oscr_d = nc.dram_tensor("oscr", [N, DHG], F16, kind="Internal")

    with tile.TileContext(nc) as tc:
        with tc.tile_pool(name="persist", bufs=1) as pp, \
             tc.tile_pool(name="ptp", bufs=3) as ptp, \
             tc.tile_pool(name="osb", bufs=2) as osb, \
             tc.tile_pool(name="otp", bufs=2) as otp, \
             tc.tile_pool(name="od", bufs=4) as od, \
             tc.tile_pool(name="psS", bufs=2, space="PSUM") as psS, \
             tc.tile_pool(name="psV", bufs=1, space="PSUM") as psV, \
             tc.tile_pool(name="psF", bufs=2, space="PSUM") as psF:
            wk_sb = pp.tile([P, KO, DHG], F16)
            wv_sb = pp.tile([P, KO, DHG], F16)
            wq_sb = pp.tile([P, KO, DHG], F16)
            wo_sb = pp.tile([P, DO, E], F16)
            bo_sb = pp.tile([1, E], F32)
            bias_sb = pp.tile([P, E], F32)
            rec_sb = pp.tile([P, QC, HL, NC], F32)   # 1/rowsum per (qc, h, nci)
            ctxT = pp.tile([P, KO, M], F16)
            xT = pp.tile([P, KO, N], F16)
            kT = pp.tile([P, DO, M], F16)    # K.T [dhg, m]
            qT = pp.tile([P, DO, N], F16)    # Q.T [dhg, n]
            v_sb = pp.tile([P, MT, HL, DH + 1], F16)  # V + ones col per head

            nc.sync.dma_start(wk_sb[:], wk_d.rearrange("(ko p) d -> p ko d", p=P))
            for ms in range(2):
                for ko in range(KO):
                    nc.sync.dma_start_transpose(
                        ctxT[:, ko, ms * 512:(ms + 1) * 512],
                        ctx_d[ms * 512:(ms + 1) * 512, ko * P:(ko + 1) * P],
                    )
            nc.sync.dma_start(wv_sb[:], wv_d.rearrange("(ko p) d -> p ko d", p=P))
            nc.sync.dma_start(wq_sb[:], wq_d.rearrange("(ko p) d -> p ko d", p=P))
            for qc in range(QC):
                for ko in range(KO):
                    nc.sync.dma_start_transpose(
                        xT[:, ko, qc * 512:(qc + 1) * 512],
                        x_d[qc * 512:(qc + 1) * 512, ko * P:(ko + 1) * P],
                    )
            nc.sync.dma_start(wo_sb[:], wo_d.rearrange("(do p) e -> p do e", p=P))
            nc.sync.dma_start(bo_sb[:], bo_d[:])
            nc.gpsimd.partition_broadcast(bias_sb[:], bo_sb[:])
            nc.vector.memset(v_sb[:, :, :, DH], 1.0)

            # ---------------- phase A: K.T and V projections -------------
            for do in range(DO):
                s = psS.tile([P, 2, 512], F32, tag="s", name=f"ks_{do}")
                for ms in range(2):
                    for ko in range(KO):
                        nc.tensor.matmul(
                            s[:, ms],
                            wk_sb[:, ko, do * P:(do + 1) * P],
                            ctxT[:, ko, ms * 512:(ms + 1) * 512],
                            start=(ko == 0), stop=(ko == KO - 1),
                        )
                nc.vector.tensor_copy(kT[:, do, :], s[:])
            for mp in range(MT // 2):
                s = psS.tile([P, 2, 512], F32, tag="s", name=f"vs_{mp}")
                for k2 in range(2):
                    mo = 2 * mp + k2
                    for ko in range(KO):
                        nc.tensor.matmul(
                            s[:, k2],
                            ctxT[:, ko, mo * P:(mo + 1) * P],
                            wv_sb[:, ko, :],
                            start=(ko == 0), stop=(ko == KO - 1),
                        )
                nc.vector.tensor_copy(
                    v_sb[:, 2 * mp:2 * mp + 2, :, 0:DH],
                    s[:].rearrange("p a (h d) -> p a h d", h=HL),
                )

            def q_proj(qc):
                q0 = qc * 512
                for dp in range(DO // 2):
                    s = psS.tile([P, 2, 512], F32, tag="s", name=f"qs_{qc}_{dp}")
                    for k2 in range(2):
                        do = 2 * dp + k2
                        for ko in range(KO):
                            nc.tensor.matmul(
                                s[:, k2],
                                wq_sb[:, ko, do * P:(do + 1) * P],
                                xT[:, ko, q0:q0 + 512],
                                start=(ko == 0), stop=(ko == KO - 1),
                            )
                    nc.vector.tensor_copy(qT[:, 2 * dp:2 * dp + 2, q0:q0 + 512], s[:])

            q_proj(0)

            oT_tiles = {}

            def out_proj(qc, lo, hi):
                q0 = qc * 512
                oT = oT_tiles[qc]
                for t in range(lo, hi):
                    nci, ec = t // EC, t % EC
                    fps = psF.tile([P, 512], F32, tag="f", name=f"f_{qc}_{t}")
                    for do in range(DO):
                        nc.tensor.matmul(
                            fps[:],
                            oT[:, do, nci * P:(nci + 1) * P],
                            wo_sb[:, do, ec * 512:(ec + 1) * 512],
                            start=(do == 0), stop=(do == DO - 1),
                        )
                    ot = od.tile([P, 512], F16, tag="ob", name=f"ob_{qc}_{t}")
                    nc.vector.tensor_tensor(
                        ot[:], fps[:], bias_sb[:, ec * 512:(ec + 1) * 512],
                        mybir.AluOpType.add,
                    )
                    nc.gpsimd.dma_start(
                        out_d[q0 + nci * P:q0 + (nci + 1) * P,
                              ec * 512:(ec + 1) * 512],
                        ot[:],
                    )

            # -------- phase C: attention (+ interleaved projections) -----
            for qc in range(QC):
                q0 = qc * 512
                O_sb = osb.tile([P, HL, NC, DH], F16, tag="o", name=f"O_{qc}")
                for h in range(HL):
                    do, pb = h // 2, (h % 2) * DH
                    ptile = ptp.tile([P, MT, 512], F16, tag="pt", name=f"pt_{qc}_{h}")
                    for mp in range(MT // 2):
                        s = psS.tile([P, 2, 512], F32, tag="s", name=f"ss_{qc}_{h}_{mp}")
                        for k2 in range(2):
                            mo = 2 * mp + k2
                            nc.tensor.matmul(
                                s[:, k2],
                                kT[pb:pb + DH, do, mo * P:(mo + 1) * P],
                                qT[pb:pb + DH, do, q0:q0 + 512],
                                start=True, stop=True,
                                skip_group_check=True,
                            )
                        nc.scalar.activation(
                            ptile[:, 2 * mp:2 * mp + 2, :], s[:],
                            mybir.ActivationFunctionType.Exp, scale=SCALE,
                        )
                    for pair in range(NC // 2):
                        pv = psV.tile([P, 2, 512], F32, tag="pv",
                                      name=f"pv_{qc}_{h}_{pair}")
                        for k2 in range(2):
                            nci = 2 * pair + k2
                            for mo in range(MT):
                                nc.tensor.matmul(
                                    pv[:, k2, 0:DH + 1],
                                    ptile[:, mo, nci * P:(nci + 1) * P],
                                    v_sb[:, mo, h, :],
                                    start=(mo == 0), stop=(mo == MT - 1),
                                    skip_group_check=True,
                                )
                        rec = rec_sb[:, qc, h, 2 * pair:2 * pair + 2]
                        nc.vector.reciprocal(rec, pv[:, :, DH])
                        for k2 in range(2):
                            nci = 2 * pair + k2
                            nc.vector.tensor_scalar(
                                O_sb[:, h, nci, :],
                                pv[:, k2, 0:DH],
                                rec_sb[:, qc, h, nci:nci + 1],
                                None,
                                mybir.AluOpType.mult,
                            )
                    # per-head store of O rows to the DRAM scratch
                    nc.sync.dma_start(
                        oscr_d[q0:q0 + 512, h * DH:(h + 1) * DH]
                        .rearrange("(a pn) c -> pn a c", pn=P),
                        O_sb[:, h],
                    )
                    # interleave prior-chunk output projection + next Q proj
                    if qc > 0 and h == 1:
                        out_proj(qc - 1, 0, 4)
                    elif qc > 0 and h == 3:
                        out_proj(qc - 1, 4, 8)
                    elif h == 5 and qc + 1 < QC:
                        q_proj(qc + 1)

                oT = otp.tile([P, DO, 512], F16, tag="ot", name=f"oT_{qc}")
                nc.sync.dma_start_transpose(oT[:], oscr_d[q0:q0 + 512, :])
                oT_tiles[qc] = oT

            out_proj(QC - 1, 0, 8)
    nc.finalize()
    return nc


def _get_nc():
    if "nc" not in _CACHE:
        _CACHE["nc"] = _build()
    return _CACHE["nc"]


def kernel(x, context, Wq, Wk, Wv, Wo, bo, **extra):
    nc = _get_nc()
    B = x.shape[0]
    f16 = np.float16
    zeros_bo = np.zeros((1, E), dtype=np.float32)
    bo_full = np.ascontiguousarray(np.asarray(bo, dtype=np.float32).reshape(1, E))
    in_maps = []
    for c in range(8):
        b, g = c // 2, c % 2
        in_maps.append({
            "x": np.ascontiguousarray(x[b], dtype=f16),
            "ctx": np.ascontiguousarray(context[b], dtype=f16),
            "wq": np.ascontiguousarray(Wq[:, g * DHG:(g + 1) * DHG]).astype(f16),
            "wk": np.ascontiguousarray(Wk[:, g * DHG:(g + 1) * DHG]).astype(f16),
            "wv": np.ascontiguousarray(Wv[:, g * DHG:(g + 1) * DHG]).astype(f16),
            "wo": np.ascontiguousarray(Wo[g * DHG:(g + 1) * DHG, :]).astype(f16),
            "bo": (bo_full if g == 0 else zeros_bo),
        })
    global _last_in_maps
    _last_in_maps = in_maps
    res = run_bass_kernel_spmd(nc, in_maps, list(range(8)))
    out = np.empty((B, N, E), dtype=np.float32)
    for b in range(B):
        out[b] = res.results[2 * b]["out"].astype(np.float32) \
            + res.results[2 * b + 1]["out"].astype(np.float32)
    return out
